# revision 45
# baseline (speedup 1.0000x reference)
"""Multi-head self-attention with RoPE (causal) on 8 Trainium2 NeuronCores.

Sharding: core c -> batch b = c//4, head-group g = c%4 (heads 4g..4g+3).
Each core computes a partial output x[b] @ block of Wo; host sums the 4
partials per batch.

Design (cost-model driven):
  - q/k projections + scores in bf16 (fp8 quantization of q/k injects too
    much softmax noise); q/k live as [dk on 64-partition head-halves,
    head-pair, seq], so scores are single K=64 matmuls at
    tile_position (64*(h%2), 0).
  - RoPE: DVE stream_shuffle pair-swap; cos/sin tables carry the 1/16
    weight-scale dequant; sign folded into the sin table.
  - v and out projections in fp8-e4m3 DoubleRow (0.5 cyc/row); a bf16
    hi-precision path covers keys/queries < 512 where softmax averages
    too few keys to suppress fp8 noise.
  - exp split between ScalarE (native Exp) and DVE (Schraudolph int16
    bit-hack exp bitcast to bf16), balanced by running load counters.
  - PV uses the flipped layout out[sq, dk]: softmax denominator becomes a
    per-partition scalar (accumulated by N=1 ones-matmuls; normalize via
    activation-scale); attn tiles are PE-transposed back to [dk, sq].
  - PSUM start=True zeroes a whole 2KB bank (lazily): only the first
    write of each bank per accumulation round carries it.
"""

import ml_dtypes
import numpy as np

import concourse.bass as bass
import concourse.mybir as mybir
import concourse.tile as tile
from concourse import bacc
from concourse.bass_utils import run_bass_kernel_spmd

F32 = mybir.dt.float32
BF16 = mybir.dt.bfloat16
I16 = mybir.dt.int16
FP8 = mybir.dt.float8e4
DR = mybir.MatmulPerfMode.DoubleRow

D = 1024          # d_model
NH = 16           # total heads
DK = 64           # head dim
S = 2048          # seq len
B = 2             # batch
THETA = 10000.0
HPC = 4           # heads per core
DPC = HPC * DK    # dims per core = 256
N_CORES = 8

WS = 16.0                    # weight scale (all W * 16)
ATS = 4.0                    # attn tile scale before fp8 out-proj
OUT_DIV = WS * ATS
LOG2E = 1.4426950408889634
EXP_A = 0.125 * 128.0 * LOG2E      # schraudolph mult (incl 1/sqrt(dk))
EXP_B = 127.0 * 128.0 - 5.5        # schraudolph bias (tuned C=-5.5)
QHI = 512                          # rows/keys < QHI take the bf16 hi path
THI = QHI // 128
SWAP_MASK = [(i ^ 1) for i in range(32)]


def _emit(tc, aps):
    nc = tc.nc
    OP = mybir.AluOpType
    AF = mybir.ActivationFunctionType

    load = {"act": 0.0, "dve": 0.0, "pool": 0.0}

    with (
        tc.tile_pool(name="persist", bufs=1) as pp,
        tc.tile_pool(name="rope", bufs=8) as rp,
        tc.tile_pool(name="expp", bufs=24) as xp,
        tc.tile_pool(name="small", bufs=6) as sm,
        tc.tile_pool(name="ps512", bufs=4, space="PSUM") as ps4,
        tc.tile_pool(name="psacc", bufs=1, space="PSUM") as psA,
    ):
        xT_bf = pp.tile([128, 8, S], BF16, tag="xT_bf")
        xT8 = pp.tile([128, 4, 2, S], FP8, tag="xT8")
        wq_sb = pp.tile([128, 8, 2, 128], BF16, tag="wq")
        wk_sb = pp.tile([128, 8, 2, 128], BF16, tag="wk")
        wv8_sb = pp.tile([128, 4, 2, DPC], FP8, tag="wv8")
        wo8_sb = pp.tile([128, 2, D], FP8, tag="wo8")
        wv_hi = pp.tile([128, 8, DPC], BF16, tag="wv_hi")
        wo_hi = pp.tile([128, 2, D], BF16, tag="wo_hi")
        cos_sb = pp.tile([128, S], BF16, tag="cos")
        sin_sb = pp.tile([128, S], BF16, tag="sin")
        qT_bf = pp.tile([128, 2, S], BF16, tag="qT")
        kT_bf = pp.tile([128, 2, S], BF16, tag="kT")
        v_sb = pp.tile([128, 16, HPC, DK], BF16, tag="v")
        v_hi = pp.tile([128, THI, HPC, DK], BF16, tag="v_hi")
        attnT8 = pp.tile([128, 2, S], FP8, tag="attnT8")
        attnT_hi = pp.tile([128, 2, QHI], BF16, tag="attnT_hi")
        id_sb = pp.tile([128, 128], BF16, tag="ident")
        dm_sb = pp.tile([128, 128], BF16, tag="dmask")
        ones_sb = pp.tile([128, 1], BF16, tag="ones")
        recip_sb = pp.tile([128, 2, 16], F32, tag="recip")

        trp_ps = psA.tile([128, 2, 128], BF16, tag="trp")  # 1 bank
        den_ps = psA.tile([128, 16, 1], F32, tag="den")    # 1 bank
        at_ps = psA.tile([128, 16, DK], F32, tag="at")     # 2 banks

        # ---- input DMAs, ordered to unblock the q-projection first ----
        dma = nc.sync.dma_start
        dma(wq_sb[:], aps["wq"][:])
        dma(xT_bf[:, :, 0:512], aps["xT_bf"][:, :, 0:512])
        dma(cos_sb[:], aps["cosT"][:])
        dma(sin_sb[:], aps["sinT"][:])
        dma(wk_sb[:], aps["wk"][:])
        dma(xT_bf[:, :, 512:1024], aps["xT_bf"][:, :, 512:1024])
        dma(xT_bf[:, :, 1024:1536], aps["xT_bf"][:, :, 1024:1536])
        dma(xT_bf[:, :, 1536:2048], aps["xT_bf"][:, :, 1536:2048])
        dma(wv_hi[:], aps["wv_hi"][:])
        dma(wv8_sb[:], aps["wv8"][:])
        dma(xT8[:, :, :, 0:1024], aps["xT8"][:, :, :, 0:1024])
        dma(xT8[:, :, :, 1024:2048], aps["xT8"][:, :, :, 1024:2048])
        dma(id_sb[:], aps["ident"][:])
        dma(dm_sb[:], aps["dmask"][:])
        dma(wo8_sb[:], aps["wo8"][:])
        dma(wo_hi[:], aps["wo_hi"][:])
        nc.gpsimd.memset(ones_sb[:], 1.0)

        # ---- q/k projections (bf16) + RoPE via stream_shuffle ----
        def rope_chunk(w_sb, outT, mt, c):
            sl = slice(512 * c, 512 * (c + 1))
            ps = ps4.tile([128, 512], F32, tag="ps512", name="pj")
            for kt in range(8):
                nc.tensor.matmul(ps[:], w_sb[:, kt, mt, :], xT_bf[:, kt, sl],
                                 start=(kt == 0), stop=(kt == 7))
            sw = rp.tile([128, 512], F32, tag="sw")
            nc.vector.stream_shuffle(sw[:], ps[:], SWAP_MASK)
            load["dve"] += 660
            t1 = rp.tile([128, 512], BF16, tag="t1")
            nc.vector.tensor_tensor(t1[:], ps[:], cos_sb[:, sl], OP.mult)
            load["dve"] += 660
            t2 = rp.tile([128, 512], BF16, tag="t2")
            nc.gpsimd.tensor_tensor(t2[:], sw[:], sin_sb[:, sl], OP.mult)
            load["pool"] += 1100
            with nc.allow_low_precision(reason="bf16 qk"):
                nc.vector.tensor_tensor(outT[:, mt, sl], t1[:], t2[:], OP.add)
            load["dve"] += 250

        def v_chunk(st):
            ps = ps4.tile([128, 512], F32, tag="ps512")
            for kt2 in range(4):
                nc.tensor.matmul(ps[:, 0:DPC],
                                 xT8[:, kt2, :, 128 * st:128 * (st + 1)],
                                 wv8_sb[:, kt2, :, :],
                                 start=(kt2 == 0), stop=(kt2 == 3),
                                 perf_mode=DR)
            with nc.allow_low_precision(reason="bf16 v"):
                nc.scalar.mul(v_sb[:, st, :, :],
                              ps[:, 0:DPC].rearrange("p (h e) -> p h e", h=HPC),
                              1.0 / WS)
            load["act"] += 360

        def hi_v(st):
            ps = ps4.tile([128, 512], F32, tag="ps512")
            for kt in range(8):
                nc.tensor.matmul(ps[:, 0:DPC],
                                 xT_bf[:, kt, 128 * st:128 * (st + 1)],
                                 wv_hi[:, kt, :],
                                 start=(kt == 0), stop=(kt == 7))
            with nc.allow_low_precision(reason="bf16 v hi"):
                nc.scalar.mul(v_hi[:, st, :, :],
                              ps[:, 0:DPC].rearrange("p (h e) -> p h e", h=HPC),
                              1.0 / WS)
            load["act"] += 360

        for c in range(4):
            rope_chunk(wq_sb, qT_bf, 0, c)
            if c < 2:
                rope_chunk(wk_sb, kT_bf, 0, c)
        for st in range(THI):
            hi_v(st)
        for st in range(0, 8):
            v_chunk(st)
        fillers = []
        for c in range(2, 4):
            fillers.append((lambda c=c: rope_chunk(wk_sb, kT_bf, 0, c)))
        for st in range(8, 12):
            fillers.append((lambda st=st: v_chunk(st)))
        for c in range(4):
            fillers.append((lambda c=c: rope_chunk(wq_sb, qT_bf, 1, c)))
            fillers.append((lambda c=c: rope_chunk(wk_sb, kT_bf, 1, c)))
            if c == 0:
                for st in range(12, 16):
                    fillers.append((lambda st=st: v_chunk(st)))

        # ---- attention (sequential heads, flipped PV) ----
        def exp_chunk(sc, ex, n, diag, force_act):
            if force_act or load["act"] <= load["dve"]:
                nc.scalar.activation(ex[:, 0:n], sc[:, 0:n], AF.Exp, scale=0.125)
                load["act"] += n * 0.833 + 170
            else:
                with nc.allow_low_precision(reason="schraudolph exp"):
                    nc.vector.tensor_scalar(ex[:, 0:n].bitcast(I16), sc[:, 0:n],
                                            EXP_A, EXP_B, OP.mult, OP.add)
                load["dve"] += n * 1.042 + 170
            if diag:
                nc.gpsimd.tensor_tensor(ex[:, 0:128], ex[:, 0:128], dm_sb[:],
                                        OP.mult)
                load["pool"] += 350

        a_t_live = {}

        def norm_tile(h, qt):
            a_t = sm.tile([128, DK], BF16, tag="attn", bufs=16)
            a_t_live[(h, qt)] = a_t
            rc = recip_sb[:, h % 2, qt:qt + 1]
            src = at_ps[:, qt, :]
            if load["act"] <= load["dve"]:
                with nc.allow_low_precision(reason="bf16 attn"):
                    nc.scalar.mul(a_t[:], src, rc)
                load["act"] += 200
            else:
                with nc.allow_low_precision(reason="bf16 attn"):
                    nc.vector.tensor_scalar(a_t[:], src, rc, None, OP.mult)
                load["dve"] += 200

        def transp_one(h, qt):
            a_t = a_t_live.pop((h, qt))
            prow = slice(64 * (h % 2), 64 * (h % 2) + 64)
            nc.tensor.transpose(trp_ps[prow, qt % 2, :], a_t[:], id_sb[:])
            if qt < THI:
                dst = attnT_hi[prow, h // 2, 128 * qt:128 * (qt + 1)]
                scl = 1.0
            else:
                dst = attnT8[prow, h // 2, 128 * qt:128 * (qt + 1)]
                scl = ATS
            with nc.allow_low_precision(reason="attnT write"):
                if load["act"] <= load["dve"]:
                    nc.scalar.mul(dst, trp_ps[prow, qt % 2, :], scl)
                    load["act"] += 260
                else:
                    nc.vector.tensor_scalar(dst, trp_ps[prow, qt % 2, :], scl,
                                            None, OP.mult)
                    load["dve"] += 260

        def out_st(st, ob):
            hi = st < THI
            for ncb in range(2):
                po = ps4.tile([128, 512], F32, tag="ps512")
                if hi:
                    for kt2 in range(2):
                        nc.tensor.matmul(
                            po[:], attnT_hi[:, kt2, 128 * st:128 * (st + 1)],
                            wo_hi[:, kt2, 512 * ncb:512 * (ncb + 1)],
                            start=(kt2 == 0), stop=(kt2 == 1))
                else:
                    nc.tensor.matmul(po[:],
                                     attnT8[:, :, 128 * st:128 * (st + 1)],
                                     wo8_sb[:, :, 512 * ncb:512 * (ncb + 1)],
                                     start=True, stop=True, perf_mode=DR)
                osc = (1.0 / WS) if hi else (1.0 / OUT_DIV)
                with nc.allow_low_precision(reason="bf16 out"):
                    if load["act"] <= load["dve"]:
                        nc.scalar.mul(ob[:, st % 2, ncb, :], po[:], osc)
                        load["act"] += 580
                    else:
                        nc.vector.tensor_scalar(ob[:, st % 2, ncb, :], po[:],
                                                osc, None, OP.mult)
                        load["dve"] += 670
            if st % 2 == 1:
                dst = aps["out"][256 * (st // 2):256 * (st // 2 + 1), :]
                dma(dst.rearrange("(s p) (n c) -> p s n c", s=2, n=2), ob[:])

        pending = []

        def finish_group(h, qg):
            qsl = slice(4 * qg, 4 * qg + 4)
            with nc.allow_low_precision(reason="recip"):
                nc.vector.reciprocal(recip_sb[:, h % 2, qsl],
                                     den_ps[:, qsl, 0])
            load["dve"] += 180
            for qt in range(4 * qg, 4 * qg + 4):
                norm_tile(h, qt)
            pending.append((h, qg))

        def flush_pending():
            while pending:
                h, qg = pending.pop(0)
                ob = None
                for qt in range(4 * qg, 4 * qg + 4):
                    transp_one(h, qt)
                    if h == 3:
                        if qt % 2 == 0:
                            ob = sm.tile([128, 2, 2, 512], BF16, tag="ob",
                                         bufs=2)
                        out_st(qt, ob)

        def attention(h):
            hp = slice(64 * (h % 2), 64 * (h % 2) + 64)
            pr = h // 2
            tp = (64 * (h % 2), 0)
            for t in range(16):
                flush_pending()
                with tc.high_priority(offset=-100000):
                    for _ in range(3):
                        if fillers:
                            fillers.pop(0)()
                base = 128 * t
                L = S - base
                off = 0
                while off < L:
                    n = min(512, L - off)
                    sc = ps4.tile([128, 512], F32, tag="ps512")
                    nc.tensor.matmul(sc[:, 0:n], kT_bf[hp, pr, base:base + 128],
                                     qT_bf[hp, pr,
                                           base + off:base + off + n],
                                     start=True, stop=True, tile_position=tp)
                    ex = xp.tile([128, 512], BF16, tag="exp")
                    exp_chunk(sc, ex, n, diag=(off == 0),
                              force_act=(base + off < QHI))
                    jorder = list(range(n // 128))
                    if off == 0 and t > 0 and len(jorder) > 1:
                        jorder = jorder[1:] + [0]
                    for j in jorder:
                        qt = t + (off // 128) + j
                        exj = ex[:, 128 * j:128 * (j + 1)]
                        vsrc = (v_hi[:, t, h, :] if t < THI
                                else v_sb[:, t, h, :])
                        nc.tensor.matmul(at_ps[:, qt, :], exj, vsrc,
                                         start=(t == 0 and qt % 8 == 0),
                                         stop=(t == qt), skip_group_check=True)
                        nc.tensor.matmul(den_ps[:, qt, :], exj, ones_sb[:],
                                         start=(t == 0 and qt == 0),
                                         stop=(t == qt), skip_group_check=True)
                    off += n
                if t % 4 == 3:
                    finish_group(h, t // 4)

        for h in range(4):
            attention(h)
        flush_pending()
        while fillers:
            fillers.pop(0)()


_CACHE = {}


def _build():
    if "nc" in _CACHE:
        return _CACHE["nc"], _CACHE["aps"]
    nc = bacc.Bacc("TRN2", target_bir_lowering=False, debug=False,
                   enable_asserts=False, num_devices=N_CORES)
    aps = {
        "xT_bf": nc.dram_tensor("xT_bf", [128, 8, S], BF16, kind="ExternalInput").ap(),
        "xT8": nc.dram_tensor("xT8", [128, 4, 2, S], FP8, kind="ExternalInput").ap(),
        "wq": nc.dram_tensor("wq", [128, 8, 2, 128], BF16, kind="ExternalInput").ap(),
        "wk": nc.dram_tensor("wk", [128, 8, 2, 128], BF16, kind="ExternalInput").ap(),
        "wv8": nc.dram_tensor("wv8", [128, 4, 2, DPC], FP8, kind="ExternalInput").ap(),
        "wo8": nc.dram_tensor("wo8", [128, 2, D], FP8, kind="ExternalInput").ap(),
        "wv_hi": nc.dram_tensor("wv_hi", [128, 8, DPC], BF16, kind="ExternalInput").ap(),
        "wo_hi": nc.dram_tensor("wo_hi", [128, 2, D], BF16, kind="ExternalInput").ap(),
        "cosT": nc.dram_tensor("cosT", [128, S], BF16, kind="ExternalInput").ap(),
        "sinT": nc.dram_tensor("sinT", [128, S], BF16, kind="ExternalInput").ap(),
        "ident": nc.dram_tensor("ident", [128, 128], BF16, kind="ExternalInput").ap(),
        "dmask": nc.dram_tensor("dmask", [128, 128], BF16, kind="ExternalInput").ap(),
        "out": nc.dram_tensor("out", [S, D], BF16, kind="ExternalOutput").ap(),
    }
    with tile.TileContext(nc) as tc:
        _emit(tc, aps)
    nc.compile()
    _CACHE["nc"], _CACHE["aps"] = nc, aps
    return nc, aps


def _host_tables():
    """cos/sin [128, S] bf16: partition p = 64*(h%2)+dk, scaled by 1/16."""
    p = np.arange(128)
    pos = np.arange(S, dtype=np.float64)
    dk_idx = p % 64
    freq = THETA ** (-2.0 * (dk_idx // 2) / DK)
    ang = pos[None, :] * freq[:, None]
    cosT = (np.cos(ang) / WS).astype(np.float32)
    sgn = np.where(dk_idx % 2 == 0, -1.0, 1.0)
    sinT = (sgn[:, None] * np.sin(ang) / WS).astype(np.float32)
    return (cosT.astype(ml_dtypes.bfloat16), sinT.astype(ml_dtypes.bfloat16))


def _pack_wqk(Wc):
    """Wc [256, 1024] -> [128(p), 8(kt), 2(mt=pair), 128(m)] bf16 (x16)."""
    arr = (Wc * WS).reshape(2, 128, 8, 128)            # [mt, m, kt, p]
    return np.ascontiguousarray(
        arr.transpose(3, 2, 0, 1)).astype(ml_dtypes.bfloat16)


def make_in_maps(x, Wq, Wk, Wv, Wo):
    cosT, sinT = _host_tables()
    ident = np.eye(128, dtype=ml_dtypes.bfloat16)
    dmask = np.triu(np.ones((128, 128), ml_dtypes.bfloat16))  # keep sq >= sk
    xT_bf, xT8 = [], []
    for b in range(B):
        xt = x[b].T
        xT_bf.append(np.ascontiguousarray(
            xt.reshape(8, 128, S).transpose(1, 0, 2)).astype(ml_dtypes.bfloat16))
        xT8.append(np.ascontiguousarray(
            xt.reshape(4, 2, 128, S).transpose(2, 0, 1, 3)).astype(
                ml_dtypes.float8_e4m3))
    maps = []
    for c in range(N_CORES):
        b, g = c // 4, c % 4
        rows = slice(DPC * g, DPC * (g + 1))
        wv8 = np.ascontiguousarray(
            (Wv[rows, :].T * WS).reshape(4, 2, 128, DPC).transpose(2, 0, 1, 3))
        wo_s = np.ascontiguousarray(
            (Wo[:, rows].T * WS).reshape(2, 128, D).transpose(1, 0, 2))
        wv_hi = np.ascontiguousarray(
            (Wv[rows, :].T * WS).reshape(8, 128, DPC).transpose(1, 0, 2))
        maps.append({
            "xT_bf": xT_bf[b],
            "xT8": xT8[b],
            "wq": _pack_wqk(Wq[rows, :]),
            "wk": _pack_wqk(Wk[rows, :]),
            "wv8": wv8.astype(ml_dtypes.float8_e4m3),
            "wo8": wo_s.astype(ml_dtypes.float8_e4m3),
            "wv_hi": wv_hi.astype(ml_dtypes.bfloat16),
            "wo_hi": wo_s.astype(ml_dtypes.bfloat16),
            "cosT": cosT,
            "sinT": sinT,
            "ident": ident,
            "dmask": dmask,
        })
    return maps


def kernel(x, Wq, Wk, Wv, Wo, _trace=False, _tmpdir=None):
    x, Wq, Wk, Wv, Wo = (np.asarray(a, dtype=np.float32)
                         for a in (x, Wq, Wk, Wv, Wo))
    nc, _ = _build()
    maps = make_in_maps(x, Wq, Wk, Wv, Wo)
    res = run_bass_kernel_spmd(nc, maps, core_ids=list(range(N_CORES)),
                               trace=_trace, tmpdir=_tmpdir)
    out = np.zeros((B, S, D), np.float32)
    for c in range(N_CORES):
        out[c // 4] += res.results[c]["out"].astype(np.float32)
    if _trace:
        kernel.last_results = res
    return out


# revision 46
# speedup vs baseline: 1.0215x; 1.0215x over previous
"""Multi-head self-attention with RoPE (causal) on 8 Trainium2 NeuronCores.

Sharding: core c -> batch b = c//4, head-group g = c%4 (heads 4g..4g+3).
Each core computes a partial output x[b] @ block of Wo; host sums the 4
partials per batch.

Design (cost-model driven):
  - q/k projections + scores in bf16 (fp8 quantization of q/k injects too
    much softmax noise); q/k live as [dk on 64-partition head-halves,
    head-pair, seq], so scores are single K=64 matmuls at
    tile_position (64*(h%2), 0).
  - RoPE: DVE stream_shuffle pair-swap; cos/sin tables carry the 1/16
    weight-scale dequant; sign folded into the sin table.
  - v and out projections in fp8-e4m3 DoubleRow (0.5 cyc/row); a bf16
    hi-precision path covers keys/queries < 512 where softmax averages
    too few keys to suppress fp8 noise.
  - exp split between ScalarE (native Exp) and DVE (Schraudolph int16
    bit-hack exp bitcast to bf16), balanced by running load counters.
  - PV uses the flipped layout out[sq, dk]: softmax denominator becomes a
    per-partition scalar (accumulated by N=1 ones-matmuls; normalize via
    activation-scale); attn tiles are PE-transposed back to [dk, sq].
  - PSUM start=True zeroes a whole 2KB bank (lazily): only the first
    write of each bank per accumulation round carries it.
"""

import ml_dtypes
import numpy as np

import concourse.bass as bass
import concourse.mybir as mybir
import concourse.tile as tile
from concourse import bacc
from concourse.bass_utils import run_bass_kernel_spmd

F32 = mybir.dt.float32
BF16 = mybir.dt.bfloat16
I16 = mybir.dt.int16
FP8 = mybir.dt.float8e4
DR = mybir.MatmulPerfMode.DoubleRow

D = 1024          # d_model
NH = 16           # total heads
DK = 64           # head dim
S = 2048          # seq len
B = 2             # batch
THETA = 10000.0
HPC = 4           # heads per core
DPC = HPC * DK    # dims per core = 256
N_CORES = 8

WS = 16.0                    # weight scale (all W * 16)
ATS = 4.0                    # attn tile scale before fp8 out-proj
OUT_DIV = WS * ATS
LOG2E = 1.4426950408889634
EXP_A = 0.125 * 128.0 * LOG2E      # schraudolph mult (incl 1/sqrt(dk))
EXP_B = 127.0 * 128.0 - 5.5        # schraudolph bias (tuned C=-5.5)
QHI = 512                          # rows/keys < QHI take the bf16 hi path
THI = QHI // 128
SWAP_MASK = [(i ^ 1) for i in range(32)]


def _emit(tc, aps):
    nc = tc.nc
    OP = mybir.AluOpType
    AF = mybir.ActivationFunctionType

    load = {"act": 0.0, "dve": 0.0, "pool": 0.0}

    with (
        tc.tile_pool(name="persist", bufs=1) as pp,
        tc.tile_pool(name="rope", bufs=8) as rp,
        tc.tile_pool(name="expp", bufs=24) as xp,
        tc.tile_pool(name="small", bufs=6) as sm,
        tc.tile_pool(name="ps512", bufs=4, space="PSUM") as ps4,
        tc.tile_pool(name="psacc", bufs=1, space="PSUM") as psA,
    ):
        xT_bf = pp.tile([128, 8, S], BF16, tag="xT_bf")
        xT8 = pp.tile([128, 4, 2, S], FP8, tag="xT8")
        wq_sb = pp.tile([128, 8, 2, 128], BF16, tag="wq")
        wk_sb = pp.tile([128, 8, 2, 128], BF16, tag="wk")
        wv8_sb = pp.tile([128, 4, 2, DPC], FP8, tag="wv8")
        wo8_sb = pp.tile([128, 2, D], FP8, tag="wo8")
        wv_hi = pp.tile([128, 8, DPC], BF16, tag="wv_hi")
        wo_hi = pp.tile([128, 2, D], BF16, tag="wo_hi")
        cos_sb = pp.tile([128, S], BF16, tag="cos")
        sin_sb = pp.tile([128, S], BF16, tag="sin")
        qT_bf = pp.tile([128, 2, S], BF16, tag="qT")
        kT_bf = pp.tile([128, 2, S], BF16, tag="kT")
        v_sb = pp.tile([128, 16, HPC, DK], BF16, tag="v")
        v_hi = pp.tile([128, THI, HPC, DK], BF16, tag="v_hi")
        attnT8 = pp.tile([128, 2, S], FP8, tag="attnT8")
        attnT_hi = pp.tile([128, 2, QHI], BF16, tag="attnT_hi")
        id_sb = pp.tile([128, 128], BF16, tag="ident")
        dm_sb = pp.tile([128, 128], BF16, tag="dmask")
        ones_sb = pp.tile([128, 1], BF16, tag="ones")
        recip_sb = pp.tile([128, 2, 16], F32, tag="recip")

        trp_ps = psA.tile([128, 2, 128], BF16, tag="trp")  # 1 bank
        den_ps = psA.tile([128, 16, 1], F32, tag="den")    # 1 bank
        at_ps = psA.tile([128, 16, DK], F32, tag="at")     # 2 banks

        # ---- input DMAs, ordered to unblock the q-projection first ----
        dma = nc.sync.dma_start
        dma(wq_sb[:], aps["wq"][:])
        dma(xT_bf[:, :, 0:512], aps["xT_bf"][:, :, 0:512])
        dma(cos_sb[:], aps["cosT"][:])
        dma(sin_sb[:], aps["sinT"][:])
        dma(wk_sb[:], aps["wk"][:])
        dma(xT_bf[:, :, 512:1024], aps["xT_bf"][:, :, 512:1024])
        dma(xT_bf[:, :, 1024:1536], aps["xT_bf"][:, :, 1024:1536])
        dma(xT_bf[:, :, 1536:2048], aps["xT_bf"][:, :, 1536:2048])
        dma(wv_hi[:], aps["wv_hi"][:])
        dma(wv8_sb[:], aps["wv8"][:])
        dma(xT8[:, :, :, 0:1024], aps["xT8"][:, :, :, 0:1024])
        dma(xT8[:, :, :, 1024:2048], aps["xT8"][:, :, :, 1024:2048])
        dma(id_sb[:], aps["ident"][:])
        dma(dm_sb[:], aps["dmask"][:])
        dma(wo8_sb[:], aps["wo8"][:])
        dma(wo_hi[:], aps["wo_hi"][:])
        nc.gpsimd.memset(ones_sb[:], 1.0)

        # ---- q/k projections (bf16) + RoPE via stream_shuffle ----
        def rope_chunk(w_sb, outT, mt, c):
            sl = slice(512 * c, 512 * (c + 1))
            ps = ps4.tile([128, 512], F32, tag="ps512", name="pj")
            for kt in range(8):
                nc.tensor.matmul(ps[:], w_sb[:, kt, mt, :], xT_bf[:, kt, sl],
                                 start=(kt == 0), stop=(kt == 7))
            sw = rp.tile([128, 512], F32, tag="sw")
            nc.vector.stream_shuffle(sw[:], ps[:], SWAP_MASK)
            load["dve"] += 660
            t1 = rp.tile([128, 512], BF16, tag="t1")
            nc.vector.tensor_tensor(t1[:], ps[:], cos_sb[:, sl], OP.mult)
            load["dve"] += 660
            t2 = rp.tile([128, 512], BF16, tag="t2")
            nc.gpsimd.tensor_tensor(t2[:], sw[:], sin_sb[:, sl], OP.mult)
            load["pool"] += 1100
            with nc.allow_low_precision(reason="bf16 qk"):
                nc.vector.tensor_tensor(outT[:, mt, sl], t1[:], t2[:], OP.add)
            load["dve"] += 250

        def v_chunk(st):
            ps = ps4.tile([128, 512], F32, tag="ps512")
            for kt2 in range(4):
                nc.tensor.matmul(ps[:, 0:DPC],
                                 xT8[:, kt2, :, 128 * st:128 * (st + 1)],
                                 wv8_sb[:, kt2, :, :],
                                 start=(kt2 == 0), stop=(kt2 == 3),
                                 perf_mode=DR)
            with nc.allow_low_precision(reason="bf16 v"):
                nc.scalar.mul(v_sb[:, st, :, :],
                              ps[:, 0:DPC].rearrange("p (h e) -> p h e", h=HPC),
                              1.0 / WS)
            load["act"] += 360

        def hi_v(st):
            ps = ps4.tile([128, 512], F32, tag="ps512")
            for kt in range(8):
                nc.tensor.matmul(ps[:, 0:DPC],
                                 xT_bf[:, kt, 128 * st:128 * (st + 1)],
                                 wv_hi[:, kt, :],
                                 start=(kt == 0), stop=(kt == 7))
            with nc.allow_low_precision(reason="bf16 v hi"):
                nc.scalar.mul(v_hi[:, st, :, :],
                              ps[:, 0:DPC].rearrange("p (h e) -> p h e", h=HPC),
                              1.0 / WS)
            load["act"] += 360

        for c in range(4):
            rope_chunk(wq_sb, qT_bf, 0, c)
            if c < 2:
                rope_chunk(wk_sb, kT_bf, 0, c)
        for st in range(THI):
            hi_v(st)
        for st in range(0, 8):
            v_chunk(st)
        fillers = []
        for c in range(2, 4):
            fillers.append((lambda c=c: rope_chunk(wk_sb, kT_bf, 0, c)))
        for st in range(8, 12):
            fillers.append((lambda st=st: v_chunk(st)))
        for c in range(4):
            fillers.append((lambda c=c: rope_chunk(wq_sb, qT_bf, 1, c)))
            fillers.append((lambda c=c: rope_chunk(wk_sb, kT_bf, 1, c)))
            if c == 0:
                for st in range(12, 16):
                    fillers.append((lambda st=st: v_chunk(st)))

        # ---- attention (sequential heads, flipped PV) ----
        def exp_chunk(sc, ex, n, diag, force_act):
            if force_act or load["act"] <= load["dve"]:
                nc.scalar.activation(ex[:, 0:n], sc[:, 0:n], AF.Exp, scale=0.125)
                load["act"] += n * 0.833 + 170
            else:
                with nc.allow_low_precision(reason="schraudolph exp"):
                    nc.vector.tensor_scalar(ex[:, 0:n].bitcast(I16), sc[:, 0:n],
                                            EXP_A, EXP_B, OP.mult, OP.add)
                load["dve"] += n * 1.042 + 170
            if diag:
                nc.gpsimd.tensor_tensor(ex[:, 0:128], ex[:, 0:128], dm_sb[:],
                                        OP.mult)
                load["pool"] += 350

        a_t_live = {}

        def norm_tile(h, qt):
            a_t = sm.tile([128, DK], BF16, tag="attn", bufs=16)
            a_t_live[(h, qt)] = a_t
            rc = recip_sb[:, h % 2, qt:qt + 1]
            src = at_ps[:, qt, :]
            if load["act"] <= load["dve"]:
                with nc.allow_low_precision(reason="bf16 attn"):
                    nc.scalar.mul(a_t[:], src, rc)
                load["act"] += 200
            else:
                with nc.allow_low_precision(reason="bf16 attn"):
                    nc.vector.tensor_scalar(a_t[:], src, rc, None, OP.mult)
                load["dve"] += 200

        def transp_one(h, qt):
            a_t = a_t_live.pop((h, qt))
            prow = slice(64 * (h % 2), 64 * (h % 2) + 64)
            nc.tensor.transpose(trp_ps[prow, qt % 2, :], a_t[:], id_sb[:])
            if qt < THI:
                dst = attnT_hi[prow, h // 2, 128 * qt:128 * (qt + 1)]
                scl = 1.0
            else:
                dst = attnT8[prow, h // 2, 128 * qt:128 * (qt + 1)]
                scl = ATS
            with nc.allow_low_precision(reason="attnT write"):
                if load["act"] <= load["dve"]:
                    nc.scalar.mul(dst, trp_ps[prow, qt % 2, :], scl)
                    load["act"] += 260
                else:
                    nc.vector.tensor_scalar(dst, trp_ps[prow, qt % 2, :], scl,
                                            None, OP.mult)
                    load["dve"] += 260

        def out_st(st, ob):
            hi = st < THI
            for ncb in range(2):
                po = ps4.tile([128, 512], F32, tag="ps512")
                if hi:
                    for kt2 in range(2):
                        nc.tensor.matmul(
                            po[:], attnT_hi[:, kt2, 128 * st:128 * (st + 1)],
                            wo_hi[:, kt2, 512 * ncb:512 * (ncb + 1)],
                            start=(kt2 == 0), stop=(kt2 == 1))
                else:
                    nc.tensor.matmul(po[:],
                                     attnT8[:, :, 128 * st:128 * (st + 1)],
                                     wo8_sb[:, :, 512 * ncb:512 * (ncb + 1)],
                                     start=True, stop=True, perf_mode=DR)
                osc = (1.0 / WS) if hi else (1.0 / OUT_DIV)
                with nc.allow_low_precision(reason="bf16 out"):
                    if load["act"] <= load["dve"]:
                        nc.scalar.mul(ob[:, st % 2, ncb, :], po[:], osc)
                        load["act"] += 580
                    else:
                        nc.vector.tensor_scalar(ob[:, st % 2, ncb, :], po[:],
                                                osc, None, OP.mult)
                        load["dve"] += 670
            if st % 2 == 1:
                dst = aps["out"][256 * (st // 2):256 * (st // 2 + 1), :]
                dma(dst.rearrange("(s p) (n c) -> p s n c", s=2, n=2), ob[:])

        pending = []

        def finish_group(h, qg):
            qsl = slice(4 * qg, 4 * qg + 4)
            with nc.allow_low_precision(reason="recip"):
                nc.vector.reciprocal(recip_sb[:, h % 2, qsl],
                                     den_ps[:, qsl, 0])
            load["dve"] += 180
            for qt in range(4 * qg, 4 * qg + 4):
                norm_tile(h, qt)
            pending.append((h, qg))

        def flush_pending():
            while pending:
                h, qg = pending.pop(0)
                ob = None
                for qt in range(4 * qg, 4 * qg + 4):
                    transp_one(h, qt)
                    if h == 3:
                        if qt % 2 == 0:
                            ob = sm.tile([128, 2, 2, 512], BF16, tag="ob",
                                         bufs=2)
                        out_st(qt, ob)

        def attention(h):
            hp = slice(64 * (h % 2), 64 * (h % 2) + 64)
            pr = h // 2
            tp = (64 * (h % 2), 0)
            for t in range(16):
                flush_pending()
                with tc.high_priority(offset=-100000):
                    for _ in range(2):
                        if fillers:
                            fillers.pop(0)()
                base = 128 * t
                L = S - base
                off = 0
                while off < L:
                    n = min(512, L - off)
                    sc = ps4.tile([128, 512], F32, tag="ps512")
                    nc.tensor.matmul(sc[:, 0:n], kT_bf[hp, pr, base:base + 128],
                                     qT_bf[hp, pr,
                                           base + off:base + off + n],
                                     start=True, stop=True, tile_position=tp)
                    ex = xp.tile([128, 512], BF16, tag="exp")
                    exp_chunk(sc, ex, n, diag=(off == 0),
                              force_act=(base + off < QHI))
                    jorder = list(range(n // 128))
                    if off == 0 and t > 0 and len(jorder) > 1:
                        jorder = jorder[1:] + [0]
                    for j in jorder:
                        qt = t + (off // 128) + j
                        exj = ex[:, 128 * j:128 * (j + 1)]
                        vsrc = (v_hi[:, t, h, :] if t < THI
                                else v_sb[:, t, h, :])
                        nc.tensor.matmul(at_ps[:, qt, :], exj, vsrc,
                                         start=(t == 0 and qt % 8 == 0),
                                         stop=(t == qt), skip_group_check=True)
                        nc.tensor.matmul(den_ps[:, qt, :], exj, ones_sb[:],
                                         start=(t == 0 and qt == 0),
                                         stop=(t == qt), skip_group_check=True)
                    off += n
                if t % 4 == 3:
                    finish_group(h, t // 4)

        for h in range(4):
            attention(h)
        flush_pending()
        while fillers:
            fillers.pop(0)()


_CACHE = {}


def _build():
    if "nc" in _CACHE:
        return _CACHE["nc"], _CACHE["aps"]
    nc = bacc.Bacc("TRN2", target_bir_lowering=False, debug=False,
                   enable_asserts=False, num_devices=N_CORES)
    aps = {
        "xT_bf": nc.dram_tensor("xT_bf", [128, 8, S], BF16, kind="ExternalInput").ap(),
        "xT8": nc.dram_tensor("xT8", [128, 4, 2, S], FP8, kind="ExternalInput").ap(),
        "wq": nc.dram_tensor("wq", [128, 8, 2, 128], BF16, kind="ExternalInput").ap(),
        "wk": nc.dram_tensor("wk", [128, 8, 2, 128], BF16, kind="ExternalInput").ap(),
        "wv8": nc.dram_tensor("wv8", [128, 4, 2, DPC], FP8, kind="ExternalInput").ap(),
        "wo8": nc.dram_tensor("wo8", [128, 2, D], FP8, kind="ExternalInput").ap(),
        "wv_hi": nc.dram_tensor("wv_hi", [128, 8, DPC], BF16, kind="ExternalInput").ap(),
        "wo_hi": nc.dram_tensor("wo_hi", [128, 2, D], BF16, kind="ExternalInput").ap(),
        "cosT": nc.dram_tensor("cosT", [128, S], BF16, kind="ExternalInput").ap(),
        "sinT": nc.dram_tensor("sinT", [128, S], BF16, kind="ExternalInput").ap(),
        "ident": nc.dram_tensor("ident", [128, 128], BF16, kind="ExternalInput").ap(),
        "dmask": nc.dram_tensor("dmask", [128, 128], BF16, kind="ExternalInput").ap(),
        "out": nc.dram_tensor("out", [S, D], BF16, kind="ExternalOutput").ap(),
    }
    with tile.TileContext(nc) as tc:
        _emit(tc, aps)
    nc.compile()
    _CACHE["nc"], _CACHE["aps"] = nc, aps
    return nc, aps


def _host_tables():
    """cos/sin [128, S] bf16: partition p = 64*(h%2)+dk, scaled by 1/16."""
    p = np.arange(128)
    pos = np.arange(S, dtype=np.float64)
    dk_idx = p % 64
    freq = THETA ** (-2.0 * (dk_idx // 2) / DK)
    ang = pos[None, :] * freq[:, None]
    cosT = (np.cos(ang) / WS).astype(np.float32)
    sgn = np.where(dk_idx % 2 == 0, -1.0, 1.0)
    sinT = (sgn[:, None] * np.sin(ang) / WS).astype(np.float32)
    return (cosT.astype(ml_dtypes.bfloat16), sinT.astype(ml_dtypes.bfloat16))


def _pack_wqk(Wc):
    """Wc [256, 1024] -> [128(p), 8(kt), 2(mt=pair), 128(m)] bf16 (x16)."""
    arr = (Wc * WS).reshape(2, 128, 8, 128)            # [mt, m, kt, p]
    return np.ascontiguousarray(
        arr.transpose(3, 2, 0, 1)).astype(ml_dtypes.bfloat16)


def make_in_maps(x, Wq, Wk, Wv, Wo):
    cosT, sinT = _host_tables()
    ident = np.eye(128, dtype=ml_dtypes.bfloat16)
    dmask = np.triu(np.ones((128, 128), ml_dtypes.bfloat16))  # keep sq >= sk
    xT_bf, xT8 = [], []
    for b in range(B):
        xt = x[b].T
        xT_bf.append(np.ascontiguousarray(
            xt.reshape(8, 128, S).transpose(1, 0, 2)).astype(ml_dtypes.bfloat16))
        xT8.append(np.ascontiguousarray(
            xt.reshape(4, 2, 128, S).transpose(2, 0, 1, 3)).astype(
                ml_dtypes.float8_e4m3))
    maps = []
    for c in range(N_CORES):
        b, g = c // 4, c % 4
        rows = slice(DPC * g, DPC * (g + 1))
        wv8 = np.ascontiguousarray(
            (Wv[rows, :].T * WS).reshape(4, 2, 128, DPC).transpose(2, 0, 1, 3))
        wo_s = np.ascontiguousarray(
            (Wo[:, rows].T * WS).reshape(2, 128, D).transpose(1, 0, 2))
        wv_hi = np.ascontiguousarray(
            (Wv[rows, :].T * WS).reshape(8, 128, DPC).transpose(1, 0, 2))
        maps.append({
            "xT_bf": xT_bf[b],
            "xT8": xT8[b],
            "wq": _pack_wqk(Wq[rows, :]),
            "wk": _pack_wqk(Wk[rows, :]),
            "wv8": wv8.astype(ml_dtypes.float8_e4m3),
            "wo8": wo_s.astype(ml_dtypes.float8_e4m3),
            "wv_hi": wv_hi.astype(ml_dtypes.bfloat16),
            "wo_hi": wo_s.astype(ml_dtypes.bfloat16),
            "cosT": cosT,
            "sinT": sinT,
            "ident": ident,
            "dmask": dmask,
        })
    return maps


def kernel(x, Wq, Wk, Wv, Wo, _trace=False, _tmpdir=None):
    x, Wq, Wk, Wv, Wo = (np.asarray(a, dtype=np.float32)
                         for a in (x, Wq, Wk, Wv, Wo))
    nc, _ = _build()
    maps = make_in_maps(x, Wq, Wk, Wv, Wo)
    res = run_bass_kernel_spmd(nc, maps, core_ids=list(range(N_CORES)),
                               trace=_trace, tmpdir=_tmpdir)
    out = np.zeros((B, S, D), np.float32)
    for c in range(N_CORES):
        out[c // 4] += res.results[c]["out"].astype(np.float32)
    if _trace:
        kernel.last_results = res
    return out


# revision 47
# speedup vs baseline: 1.0226x; 1.0011x over previous
"""Multi-head self-attention with RoPE (causal) on 8 Trainium2 NeuronCores.

Sharding: core c -> batch b = c//4, head-group g = c%4 (heads 4g..4g+3).
Each core computes a partial output x[b] @ block of Wo; host sums the 4
partials per batch.

Design (cost-model driven):
  - q/k projections + scores in bf16 (fp8 quantization of q/k injects too
    much softmax noise); q/k live as [dk on 64-partition head-halves,
    head-pair, seq], so scores are single K=64 matmuls at
    tile_position (64*(h%2), 0).
  - RoPE: DVE stream_shuffle pair-swap; cos/sin tables carry the 1/16
    weight-scale dequant; sign folded into the sin table.
  - v and out projections in fp8-e4m3 DoubleRow (0.5 cyc/row); a bf16
    hi-precision path covers keys/queries < 512 where softmax averages
    too few keys to suppress fp8 noise.
  - exp split between ScalarE (native Exp) and DVE (Schraudolph int16
    bit-hack exp bitcast to bf16), balanced by running load counters.
  - PV uses the flipped layout out[sq, dk]: softmax denominator becomes a
    per-partition scalar (accumulated by N=1 ones-matmuls; normalize via
    activation-scale); attn tiles are PE-transposed back to [dk, sq].
  - PSUM start=True zeroes a whole 2KB bank (lazily): only the first
    write of each bank per accumulation round carries it.
"""

import ml_dtypes
import numpy as np

import concourse.bass as bass
import concourse.mybir as mybir
import concourse.tile as tile
from concourse import bacc
from concourse.bass_utils import run_bass_kernel_spmd

F32 = mybir.dt.float32
BF16 = mybir.dt.bfloat16
I16 = mybir.dt.int16
FP8 = mybir.dt.float8e4
DR = mybir.MatmulPerfMode.DoubleRow

D = 1024          # d_model
NH = 16           # total heads
DK = 64           # head dim
S = 2048          # seq len
B = 2             # batch
THETA = 10000.0
HPC = 4           # heads per core
DPC = HPC * DK    # dims per core = 256
N_CORES = 8

WS = 16.0                    # weight scale (all W * 16)
ATS = 4.0                    # attn tile scale before fp8 out-proj
OUT_DIV = WS * ATS
LOG2E = 1.4426950408889634
EXP_A = 0.125 * 128.0 * LOG2E      # schraudolph mult (incl 1/sqrt(dk))
EXP_B = 127.0 * 128.0 - 5.5        # schraudolph bias (tuned C=-5.5)
QHI = 512                          # rows/keys < QHI take the bf16 hi path
THI = QHI // 128
SWAP_MASK = [(i ^ 1) for i in range(32)]


def _emit(tc, aps):
    nc = tc.nc
    OP = mybir.AluOpType
    AF = mybir.ActivationFunctionType

    load = {"act": 0.0, "dve": 0.0, "pool": 0.0}

    with (
        tc.tile_pool(name="persist", bufs=1) as pp,
        tc.tile_pool(name="rope", bufs=12) as rp,
        tc.tile_pool(name="expp", bufs=24) as xp,
        tc.tile_pool(name="small", bufs=6) as sm,
        tc.tile_pool(name="ps512", bufs=4, space="PSUM") as ps4,
        tc.tile_pool(name="psacc", bufs=1, space="PSUM") as psA,
    ):
        xT_bf = pp.tile([128, 8, S], BF16, tag="xT_bf")
        xT8 = pp.tile([128, 4, 2, S], FP8, tag="xT8")
        wq_sb = pp.tile([128, 8, 2, 128], BF16, tag="wq")
        wk_sb = pp.tile([128, 8, 2, 128], BF16, tag="wk")
        wv8_sb = pp.tile([128, 4, 2, DPC], FP8, tag="wv8")
        wo8_sb = pp.tile([128, 2, D], FP8, tag="wo8")
        wv_hi = pp.tile([128, 8, DPC], BF16, tag="wv_hi")
        wo_hi = pp.tile([128, 2, D], BF16, tag="wo_hi")
        cos_sb = pp.tile([128, S], BF16, tag="cos")
        sin_sb = pp.tile([128, S], BF16, tag="sin")
        qT_bf = pp.tile([128, 2, S], BF16, tag="qT")
        kT_bf = pp.tile([128, 2, S], BF16, tag="kT")
        v_sb = pp.tile([128, 16, HPC, DK], BF16, tag="v")
        v_hi = pp.tile([128, THI, HPC, DK], BF16, tag="v_hi")
        attnT8 = pp.tile([128, 2, S], FP8, tag="attnT8")
        attnT_hi = pp.tile([128, 2, QHI], BF16, tag="attnT_hi")
        id_sb = pp.tile([128, 128], BF16, tag="ident")
        dm_sb = pp.tile([128, 128], BF16, tag="dmask")
        ones_sb = pp.tile([128, 1], BF16, tag="ones")
        recip_sb = pp.tile([128, 2, 16], F32, tag="recip")

        trp_ps = psA.tile([128, 2, 128], BF16, tag="trp")  # 1 bank
        den_ps = psA.tile([128, 16, 1], F32, tag="den")    # 1 bank
        at_ps = psA.tile([128, 16, DK], F32, tag="at")     # 2 banks

        # ---- input DMAs, ordered to unblock the q-projection first ----
        dma = nc.sync.dma_start
        dma(wq_sb[:], aps["wq"][:])
        dma(xT_bf[:, :, 0:512], aps["xT_bf"][:, :, 0:512])
        dma(cos_sb[:], aps["cosT"][:])
        dma(sin_sb[:], aps["sinT"][:])
        dma(wk_sb[:], aps["wk"][:])
        dma(xT_bf[:, :, 512:1024], aps["xT_bf"][:, :, 512:1024])
        dma(xT_bf[:, :, 1024:1536], aps["xT_bf"][:, :, 1024:1536])
        dma(xT_bf[:, :, 1536:2048], aps["xT_bf"][:, :, 1536:2048])
        dma(wv_hi[:], aps["wv_hi"][:])
        dma(wv8_sb[:], aps["wv8"][:])
        dma(xT8[:, :, :, 0:1024], aps["xT8"][:, :, :, 0:1024])
        dma(xT8[:, :, :, 1024:2048], aps["xT8"][:, :, :, 1024:2048])
        dma(id_sb[:], aps["ident"][:])
        dma(dm_sb[:], aps["dmask"][:])
        dma(wo8_sb[:], aps["wo8"][:])
        dma(wo_hi[:], aps["wo_hi"][:])
        nc.gpsimd.memset(ones_sb[:], 1.0)

        # ---- q/k projections (bf16) + RoPE via stream_shuffle ----
        def rope_chunk(w_sb, outT, mt, c):
            sl = slice(512 * c, 512 * (c + 1))
            ps = ps4.tile([128, 512], F32, tag="ps512", name="pj")
            for kt in range(8):
                nc.tensor.matmul(ps[:], w_sb[:, kt, mt, :], xT_bf[:, kt, sl],
                                 start=(kt == 0), stop=(kt == 7))
            sw = rp.tile([128, 512], F32, tag="sw")
            nc.vector.stream_shuffle(sw[:], ps[:], SWAP_MASK)
            load["dve"] += 660
            t1 = rp.tile([128, 512], BF16, tag="t1")
            nc.vector.tensor_tensor(t1[:], ps[:], cos_sb[:, sl], OP.mult)
            load["dve"] += 660
            t2 = rp.tile([128, 512], BF16, tag="t2")
            nc.gpsimd.tensor_tensor(t2[:], sw[:], sin_sb[:, sl], OP.mult)
            load["pool"] += 1100
            with nc.allow_low_precision(reason="bf16 qk"):
                nc.vector.tensor_tensor(outT[:, mt, sl], t1[:], t2[:], OP.add)
            load["dve"] += 250

        def v_chunk(st):
            ps = ps4.tile([128, 512], F32, tag="ps512")
            for kt2 in range(4):
                nc.tensor.matmul(ps[:, 0:DPC],
                                 xT8[:, kt2, :, 128 * st:128 * (st + 1)],
                                 wv8_sb[:, kt2, :, :],
                                 start=(kt2 == 0), stop=(kt2 == 3),
                                 perf_mode=DR)
            with nc.allow_low_precision(reason="bf16 v"):
                nc.scalar.mul(v_sb[:, st, :, :],
                              ps[:, 0:DPC].rearrange("p (h e) -> p h e", h=HPC),
                              1.0 / WS)
            load["act"] += 360

        def hi_v(st):
            ps = ps4.tile([128, 512], F32, tag="ps512")
            for kt in range(8):
                nc.tensor.matmul(ps[:, 0:DPC],
                                 xT_bf[:, kt, 128 * st:128 * (st + 1)],
                                 wv_hi[:, kt, :],
                                 start=(kt == 0), stop=(kt == 7))
            with nc.allow_low_precision(reason="bf16 v hi"):
                nc.scalar.mul(v_hi[:, st, :, :],
                              ps[:, 0:DPC].rearrange("p (h e) -> p h e", h=HPC),
                              1.0 / WS)
            load["act"] += 360

        for c in range(4):
            rope_chunk(wq_sb, qT_bf, 0, c)
            if c < 2:
                rope_chunk(wk_sb, kT_bf, 0, c)
        for st in range(THI):
            hi_v(st)
        for st in range(0, 8):
            v_chunk(st)
        fillers = []
        for c in range(2, 4):
            fillers.append((lambda c=c: rope_chunk(wk_sb, kT_bf, 0, c)))
        for st in range(8, 12):
            fillers.append((lambda st=st: v_chunk(st)))
        for c in range(4):
            fillers.append((lambda c=c: rope_chunk(wq_sb, qT_bf, 1, c)))
            fillers.append((lambda c=c: rope_chunk(wk_sb, kT_bf, 1, c)))
            if c == 0:
                for st in range(12, 16):
                    fillers.append((lambda st=st: v_chunk(st)))

        # ---- attention (sequential heads, flipped PV) ----
        def exp_chunk(sc, ex, n, diag, force_act):
            if force_act or load["act"] <= load["dve"]:
                nc.scalar.activation(ex[:, 0:n], sc[:, 0:n], AF.Exp, scale=0.125)
                load["act"] += n * 0.833 + 170
            else:
                with nc.allow_low_precision(reason="schraudolph exp"):
                    nc.vector.tensor_scalar(ex[:, 0:n].bitcast(I16), sc[:, 0:n],
                                            EXP_A, EXP_B, OP.mult, OP.add)
                load["dve"] += n * 1.042 + 170
            if diag:
                nc.gpsimd.tensor_tensor(ex[:, 0:128], ex[:, 0:128], dm_sb[:],
                                        OP.mult)
                load["pool"] += 350

        a_t_live = {}

        def norm_tile(h, qt):
            a_t = sm.tile([128, DK], BF16, tag="attn", bufs=20)
            a_t_live[(h, qt)] = a_t
            rc = recip_sb[:, h % 2, qt:qt + 1]
            src = at_ps[:, qt, :]
            if load["act"] <= load["dve"]:
                with nc.allow_low_precision(reason="bf16 attn"):
                    nc.scalar.mul(a_t[:], src, rc)
                load["act"] += 200
            else:
                with nc.allow_low_precision(reason="bf16 attn"):
                    nc.vector.tensor_scalar(a_t[:], src, rc, None, OP.mult)
                load["dve"] += 200

        def transp_one(h, qt):
            a_t = a_t_live.pop((h, qt))
            prow = slice(64 * (h % 2), 64 * (h % 2) + 64)
            nc.tensor.transpose(trp_ps[prow, qt % 2, :], a_t[:], id_sb[:])
            if qt < THI:
                dst = attnT_hi[prow, h // 2, 128 * qt:128 * (qt + 1)]
                scl = 1.0
            else:
                dst = attnT8[prow, h // 2, 128 * qt:128 * (qt + 1)]
                scl = ATS
            with nc.allow_low_precision(reason="attnT write"):
                if load["act"] <= load["dve"]:
                    nc.scalar.mul(dst, trp_ps[prow, qt % 2, :], scl)
                    load["act"] += 260
                else:
                    nc.vector.tensor_scalar(dst, trp_ps[prow, qt % 2, :], scl,
                                            None, OP.mult)
                    load["dve"] += 260

        def out_st(st, ob):
            hi = st < THI
            for ncb in range(2):
                po = ps4.tile([128, 512], F32, tag="ps512")
                if hi:
                    for kt2 in range(2):
                        nc.tensor.matmul(
                            po[:], attnT_hi[:, kt2, 128 * st:128 * (st + 1)],
                            wo_hi[:, kt2, 512 * ncb:512 * (ncb + 1)],
                            start=(kt2 == 0), stop=(kt2 == 1))
                else:
                    nc.tensor.matmul(po[:],
                                     attnT8[:, :, 128 * st:128 * (st + 1)],
                                     wo8_sb[:, :, 512 * ncb:512 * (ncb + 1)],
                                     start=True, stop=True, perf_mode=DR)
                osc = (1.0 / WS) if hi else (1.0 / OUT_DIV)
                with nc.allow_low_precision(reason="bf16 out"):
                    if load["act"] <= load["dve"]:
                        nc.scalar.mul(ob[:, st % 2, ncb, :], po[:], osc)
                        load["act"] += 580
                    else:
                        nc.vector.tensor_scalar(ob[:, st % 2, ncb, :], po[:],
                                                osc, None, OP.mult)
                        load["dve"] += 670
            if st % 2 == 1:
                dst = aps["out"][256 * (st // 2):256 * (st // 2 + 1), :]
                dma(dst.rearrange("(s p) (n c) -> p s n c", s=2, n=2), ob[:])

        pending = []

        def finish_group(h, qg):
            qsl = slice(4 * qg, 4 * qg + 4)
            with nc.allow_low_precision(reason="recip"):
                nc.vector.reciprocal(recip_sb[:, h % 2, qsl],
                                     den_ps[:, qsl, 0])
            load["dve"] += 180
            for qt in range(4 * qg, 4 * qg + 4):
                norm_tile(h, qt)
            pending.append((h, qg))

        def flush_pending():
            while pending:
                h, qg = pending.pop(0)
                ob = None
                for qt in range(4 * qg, 4 * qg + 4):
                    transp_one(h, qt)
                    if h == 3:
                        if qt % 2 == 0:
                            ob = sm.tile([128, 2, 2, 512], BF16, tag="ob",
                                         bufs=2)
                        out_st(qt, ob)

        def attention(h):
            hp = slice(64 * (h % 2), 64 * (h % 2) + 64)
            pr = h // 2
            tp = (64 * (h % 2), 0)
            for t in range(16):
                flush_pending()
                with tc.high_priority(offset=-100000):
                    for _ in range(2):
                        if fillers:
                            fillers.pop(0)()
                base = 128 * t
                L = S - base
                off = 0
                while off < L:
                    n = min(512, L - off)
                    sc = ps4.tile([128, 512], F32, tag="ps512")
                    nc.tensor.matmul(sc[:, 0:n], kT_bf[hp, pr, base:base + 128],
                                     qT_bf[hp, pr,
                                           base + off:base + off + n],
                                     start=True, stop=True, tile_position=tp)
                    ex = xp.tile([128, 512], BF16, tag="exp")
                    exp_chunk(sc, ex, n, diag=(off == 0),
                              force_act=(base + off < QHI))
                    jorder = list(range(n // 128))
                    if off == 0 and t > 0 and len(jorder) > 1:
                        jorder = jorder[1:] + [0]
                    for j in jorder:
                        qt = t + (off // 128) + j
                        exj = ex[:, 128 * j:128 * (j + 1)]
                        vsrc = (v_hi[:, t, h, :] if t < THI
                                else v_sb[:, t, h, :])
                        nc.tensor.matmul(at_ps[:, qt, :], exj, vsrc,
                                         start=(t == 0 and qt % 8 == 0),
                                         stop=(t == qt), skip_group_check=True)
                        nc.tensor.matmul(den_ps[:, qt, :], exj, ones_sb[:],
                                         start=(t == 0 and qt == 0),
                                         stop=(t == qt), skip_group_check=True)
                    off += n
                if t % 4 == 3:
                    finish_group(h, t // 4)

        for h in range(4):
            attention(h)
        flush_pending()
        while fillers:
            fillers.pop(0)()


_CACHE = {}


def _build():
    if "nc" in _CACHE:
        return _CACHE["nc"], _CACHE["aps"]
    nc = bacc.Bacc("TRN2", target_bir_lowering=False, debug=False,
                   enable_asserts=False, num_devices=N_CORES)
    aps = {
        "xT_bf": nc.dram_tensor("xT_bf", [128, 8, S], BF16, kind="ExternalInput").ap(),
        "xT8": nc.dram_tensor("xT8", [128, 4, 2, S], FP8, kind="ExternalInput").ap(),
        "wq": nc.dram_tensor("wq", [128, 8, 2, 128], BF16, kind="ExternalInput").ap(),
        "wk": nc.dram_tensor("wk", [128, 8, 2, 128], BF16, kind="ExternalInput").ap(),
        "wv8": nc.dram_tensor("wv8", [128, 4, 2, DPC], FP8, kind="ExternalInput").ap(),
        "wo8": nc.dram_tensor("wo8", [128, 2, D], FP8, kind="ExternalInput").ap(),
        "wv_hi": nc.dram_tensor("wv_hi", [128, 8, DPC], BF16, kind="ExternalInput").ap(),
        "wo_hi": nc.dram_tensor("wo_hi", [128, 2, D], BF16, kind="ExternalInput").ap(),
        "cosT": nc.dram_tensor("cosT", [128, S], BF16, kind="ExternalInput").ap(),
        "sinT": nc.dram_tensor("sinT", [128, S], BF16, kind="ExternalInput").ap(),
        "ident": nc.dram_tensor("ident", [128, 128], BF16, kind="ExternalInput").ap(),
        "dmask": nc.dram_tensor("dmask", [128, 128], BF16, kind="ExternalInput").ap(),
        "out": nc.dram_tensor("out", [S, D], BF16, kind="ExternalOutput").ap(),
    }
    with tile.TileContext(nc) as tc:
        _emit(tc, aps)
    nc.compile()
    _CACHE["nc"], _CACHE["aps"] = nc, aps
    return nc, aps


def _host_tables():
    """cos/sin [128, S] bf16: partition p = 64*(h%2)+dk, scaled by 1/16."""
    p = np.arange(128)
    pos = np.arange(S, dtype=np.float64)
    dk_idx = p % 64
    freq = THETA ** (-2.0 * (dk_idx // 2) / DK)
    ang = pos[None, :] * freq[:, None]
    cosT = (np.cos(ang) / WS).astype(np.float32)
    sgn = np.where(dk_idx % 2 == 0, -1.0, 1.0)
    sinT = (sgn[:, None] * np.sin(ang) / WS).astype(np.float32)
    return (cosT.astype(ml_dtypes.bfloat16), sinT.astype(ml_dtypes.bfloat16))


def _pack_wqk(Wc):
    """Wc [256, 1024] -> [128(p), 8(kt), 2(mt=pair), 128(m)] bf16 (x16)."""
    arr = (Wc * WS).reshape(2, 128, 8, 128)            # [mt, m, kt, p]
    return np.ascontiguousarray(
        arr.transpose(3, 2, 0, 1)).astype(ml_dtypes.bfloat16)


def make_in_maps(x, Wq, Wk, Wv, Wo):
    cosT, sinT = _host_tables()
    ident = np.eye(128, dtype=ml_dtypes.bfloat16)
    dmask = np.triu(np.ones((128, 128), ml_dtypes.bfloat16))  # keep sq >= sk
    xT_bf, xT8 = [], []
    for b in range(B):
        xt = x[b].T
        xT_bf.append(np.ascontiguousarray(
            xt.reshape(8, 128, S).transpose(1, 0, 2)).astype(ml_dtypes.bfloat16))
        xT8.append(np.ascontiguousarray(
            xt.reshape(4, 2, 128, S).transpose(2, 0, 1, 3)).astype(
                ml_dtypes.float8_e4m3))
    maps = []
    for c in range(N_CORES):
        b, g = c // 4, c % 4
        rows = slice(DPC * g, DPC * (g + 1))
        wv8 = np.ascontiguousarray(
            (Wv[rows, :].T * WS).reshape(4, 2, 128, DPC).transpose(2, 0, 1, 3))
        wo_s = np.ascontiguousarray(
            (Wo[:, rows].T * WS).reshape(2, 128, D).transpose(1, 0, 2))
        wv_hi = np.ascontiguousarray(
            (Wv[rows, :].T * WS).reshape(8, 128, DPC).transpose(1, 0, 2))
        maps.append({
            "xT_bf": xT_bf[b],
            "xT8": xT8[b],
            "wq": _pack_wqk(Wq[rows, :]),
            "wk": _pack_wqk(Wk[rows, :]),
            "wv8": wv8.astype(ml_dtypes.float8_e4m3),
            "wo8": wo_s.astype(ml_dtypes.float8_e4m3),
            "wv_hi": wv_hi.astype(ml_dtypes.bfloat16),
            "wo_hi": wo_s.astype(ml_dtypes.bfloat16),
            "cosT": cosT,
            "sinT": sinT,
            "ident": ident,
            "dmask": dmask,
        })
    return maps


def kernel(x, Wq, Wk, Wv, Wo, _trace=False, _tmpdir=None):
    x, Wq, Wk, Wv, Wo = (np.asarray(a, dtype=np.float32)
                         for a in (x, Wq, Wk, Wv, Wo))
    nc, _ = _build()
    maps = make_in_maps(x, Wq, Wk, Wv, Wo)
    res = run_bass_kernel_spmd(nc, maps, core_ids=list(range(N_CORES)),
                               trace=_trace, tmpdir=_tmpdir)
    out = np.zeros((B, S, D), np.float32)
    for c in range(N_CORES):
        out[c // 4] += res.results[c]["out"].astype(np.float32)
    if _trace:
        kernel.last_results = res
    return out


# revision 48
# speedup vs baseline: 1.0238x; 1.0011x over previous
"""Multi-head self-attention with RoPE (causal) on 8 Trainium2 NeuronCores.

Sharding: core c -> batch b = c//4, head-group g = c%4 (heads 4g..4g+3).
Each core computes a partial output x[b] @ block of Wo; host sums the 4
partials per batch.

Design (cost-model driven):
  - q/k projections + scores in bf16 (fp8 quantization of q/k injects too
    much softmax noise); q/k live as [dk on 64-partition head-halves,
    head-pair, seq], so scores are single K=64 matmuls at
    tile_position (64*(h%2), 0).
  - RoPE: DVE stream_shuffle pair-swap; cos/sin tables carry the 1/16
    weight-scale dequant; sign folded into the sin table.
  - v and out projections in fp8-e4m3 DoubleRow (0.5 cyc/row); a bf16
    hi-precision path covers keys/queries < 512 where softmax averages
    too few keys to suppress fp8 noise.
  - exp split between ScalarE (native Exp) and DVE (Schraudolph int16
    bit-hack exp bitcast to bf16), balanced by running load counters.
  - PV uses the flipped layout out[sq, dk]: softmax denominator becomes a
    per-partition scalar (accumulated by N=1 ones-matmuls; normalize via
    activation-scale); attn tiles are PE-transposed back to [dk, sq].
  - PSUM start=True zeroes a whole 2KB bank (lazily): only the first
    write of each bank per accumulation round carries it.
"""

import ml_dtypes
import numpy as np

import concourse.bass as bass
import concourse.mybir as mybir
import concourse.tile as tile
from concourse import bacc
from concourse.bass_utils import run_bass_kernel_spmd

F32 = mybir.dt.float32
BF16 = mybir.dt.bfloat16
I16 = mybir.dt.int16
FP8 = mybir.dt.float8e4
DR = mybir.MatmulPerfMode.DoubleRow

D = 1024          # d_model
NH = 16           # total heads
DK = 64           # head dim
S = 2048          # seq len
B = 2             # batch
THETA = 10000.0
HPC = 4           # heads per core
DPC = HPC * DK    # dims per core = 256
N_CORES = 8

WS = 16.0                    # weight scale (all W * 16)
ATS = 4.0                    # attn tile scale before fp8 out-proj
OUT_DIV = WS * ATS
LOG2E = 1.4426950408889634
EXP_A = 0.125 * 128.0 * LOG2E      # schraudolph mult (incl 1/sqrt(dk))
EXP_B = 127.0 * 128.0 - 5.5        # schraudolph bias (tuned C=-5.5)
QHI = 512                          # rows/keys < QHI take the bf16 hi path
THI = QHI // 128
SWAP_MASK = [(i ^ 1) for i in range(32)]


def _emit(tc, aps):
    nc = tc.nc
    OP = mybir.AluOpType
    AF = mybir.ActivationFunctionType

    load = {"act": 0.0, "dve": 0.0, "pool": 0.0}

    with (
        tc.tile_pool(name="persist", bufs=1) as pp,
        tc.tile_pool(name="rope", bufs=16) as rp,
        tc.tile_pool(name="expp", bufs=24) as xp,
        tc.tile_pool(name="small", bufs=6) as sm,
        tc.tile_pool(name="ps512", bufs=4, space="PSUM") as ps4,
        tc.tile_pool(name="psacc", bufs=1, space="PSUM") as psA,
    ):
        xT_bf = pp.tile([128, 8, S], BF16, tag="xT_bf")
        xT8 = pp.tile([128, 4, 2, S], FP8, tag="xT8")
        wq_sb = pp.tile([128, 8, 2, 128], BF16, tag="wq")
        wk_sb = pp.tile([128, 8, 2, 128], BF16, tag="wk")
        wv8_sb = pp.tile([128, 4, 2, DPC], FP8, tag="wv8")
        wo8_sb = pp.tile([128, 2, D], FP8, tag="wo8")
        wv_hi = pp.tile([128, 8, DPC], BF16, tag="wv_hi")
        wo_hi = pp.tile([128, 2, D], BF16, tag="wo_hi")
        cos_sb = pp.tile([128, S], BF16, tag="cos")
        sin_sb = pp.tile([128, S], BF16, tag="sin")
        qT_bf = pp.tile([128, 2, S], BF16, tag="qT")
        kT_bf = pp.tile([128, 2, S], BF16, tag="kT")
        v_sb = pp.tile([128, 16, HPC, DK], BF16, tag="v")
        v_hi = pp.tile([128, THI, HPC, DK], BF16, tag="v_hi")
        attnT8 = pp.tile([128, 2, S], FP8, tag="attnT8")
        attnT_hi = pp.tile([128, 2, QHI], BF16, tag="attnT_hi")
        id_sb = pp.tile([128, 128], BF16, tag="ident")
        dm_sb = pp.tile([128, 128], BF16, tag="dmask")
        ones_sb = pp.tile([128, 1], BF16, tag="ones")
        recip_sb = pp.tile([128, 2, 16], F32, tag="recip")

        trp_ps = psA.tile([128, 2, 128], BF16, tag="trp")  # 1 bank
        den_ps = psA.tile([128, 16, 1], F32, tag="den")    # 1 bank
        at_ps = psA.tile([128, 16, DK], F32, tag="at")     # 2 banks

        # ---- input DMAs, ordered to unblock the q-projection first ----
        dma = nc.sync.dma_start
        dma(wq_sb[:], aps["wq"][:])
        dma(xT_bf[:, :, 0:512], aps["xT_bf"][:, :, 0:512])
        dma(cos_sb[:], aps["cosT"][:])
        dma(sin_sb[:], aps["sinT"][:])
        dma(wk_sb[:], aps["wk"][:])
        dma(xT_bf[:, :, 512:1024], aps["xT_bf"][:, :, 512:1024])
        dma(xT_bf[:, :, 1024:1536], aps["xT_bf"][:, :, 1024:1536])
        dma(xT_bf[:, :, 1536:2048], aps["xT_bf"][:, :, 1536:2048])
        dma(wv_hi[:], aps["wv_hi"][:])
        dma(wv8_sb[:], aps["wv8"][:])
        dma(xT8[:, :, :, 0:1024], aps["xT8"][:, :, :, 0:1024])
        dma(xT8[:, :, :, 1024:2048], aps["xT8"][:, :, :, 1024:2048])
        dma(id_sb[:], aps["ident"][:])
        dma(dm_sb[:], aps["dmask"][:])
        dma(wo8_sb[:], aps["wo8"][:])
        dma(wo_hi[:], aps["wo_hi"][:])
        nc.gpsimd.memset(ones_sb[:], 1.0)

        # ---- q/k projections (bf16) + RoPE via stream_shuffle ----
        def rope_chunk(w_sb, outT, mt, c):
            sl = slice(512 * c, 512 * (c + 1))
            ps = ps4.tile([128, 512], F32, tag="ps512", name="pj")
            for kt in range(8):
                nc.tensor.matmul(ps[:], w_sb[:, kt, mt, :], xT_bf[:, kt, sl],
                                 start=(kt == 0), stop=(kt == 7))
            sw = rp.tile([128, 512], F32, tag="sw")
            nc.vector.stream_shuffle(sw[:], ps[:], SWAP_MASK)
            load["dve"] += 660
            t1 = rp.tile([128, 512], BF16, tag="t1")
            nc.vector.tensor_tensor(t1[:], ps[:], cos_sb[:, sl], OP.mult)
            load["dve"] += 660
            t2 = rp.tile([128, 512], BF16, tag="t2")
            nc.gpsimd.tensor_tensor(t2[:], sw[:], sin_sb[:, sl], OP.mult)
            load["pool"] += 1100
            with nc.allow_low_precision(reason="bf16 qk"):
                nc.vector.tensor_tensor(outT[:, mt, sl], t1[:], t2[:], OP.add)
            load["dve"] += 250

        def v_chunk(st):
            ps = ps4.tile([128, 512], F32, tag="ps512")
            for kt2 in range(4):
                nc.tensor.matmul(ps[:, 0:DPC],
                                 xT8[:, kt2, :, 128 * st:128 * (st + 1)],
                                 wv8_sb[:, kt2, :, :],
                                 start=(kt2 == 0), stop=(kt2 == 3),
                                 perf_mode=DR)
            with nc.allow_low_precision(reason="bf16 v"):
                nc.scalar.mul(v_sb[:, st, :, :],
                              ps[:, 0:DPC].rearrange("p (h e) -> p h e", h=HPC),
                              1.0 / WS)
            load["act"] += 360

        def hi_v(st):
            ps = ps4.tile([128, 512], F32, tag="ps512")
            for kt in range(8):
                nc.tensor.matmul(ps[:, 0:DPC],
                                 xT_bf[:, kt, 128 * st:128 * (st + 1)],
                                 wv_hi[:, kt, :],
                                 start=(kt == 0), stop=(kt == 7))
            with nc.allow_low_precision(reason="bf16 v hi"):
                nc.scalar.mul(v_hi[:, st, :, :],
                              ps[:, 0:DPC].rearrange("p (h e) -> p h e", h=HPC),
                              1.0 / WS)
            load["act"] += 360

        for c in range(4):
            rope_chunk(wq_sb, qT_bf, 0, c)
            if c < 2:
                rope_chunk(wk_sb, kT_bf, 0, c)
        for st in range(THI):
            hi_v(st)
        for st in range(0, 8):
            v_chunk(st)
        fillers = []
        for c in range(2, 4):
            fillers.append((lambda c=c: rope_chunk(wk_sb, kT_bf, 0, c)))
        for st in range(8, 12):
            fillers.append((lambda st=st: v_chunk(st)))
        for c in range(4):
            fillers.append((lambda c=c: rope_chunk(wq_sb, qT_bf, 1, c)))
            fillers.append((lambda c=c: rope_chunk(wk_sb, kT_bf, 1, c)))
            if c == 0:
                for st in range(12, 16):
                    fillers.append((lambda st=st: v_chunk(st)))

        # ---- attention (sequential heads, flipped PV) ----
        def exp_chunk(sc, ex, n, diag, force_act):
            if force_act or load["act"] <= load["dve"]:
                nc.scalar.activation(ex[:, 0:n], sc[:, 0:n], AF.Exp, scale=0.125)
                load["act"] += n * 0.833 + 170
            else:
                with nc.allow_low_precision(reason="schraudolph exp"):
                    nc.vector.tensor_scalar(ex[:, 0:n].bitcast(I16), sc[:, 0:n],
                                            EXP_A, EXP_B, OP.mult, OP.add)
                load["dve"] += n * 1.042 + 170
            if diag:
                nc.gpsimd.tensor_tensor(ex[:, 0:128], ex[:, 0:128], dm_sb[:],
                                        OP.mult)
                load["pool"] += 350

        a_t_live = {}

        def norm_tile(h, qt):
            a_t = sm.tile([128, DK], BF16, tag="attn", bufs=24)
            a_t_live[(h, qt)] = a_t
            rc = recip_sb[:, h % 2, qt:qt + 1]
            src = at_ps[:, qt, :]
            if load["act"] <= load["dve"]:
                with nc.allow_low_precision(reason="bf16 attn"):
                    nc.scalar.mul(a_t[:], src, rc)
                load["act"] += 200
            else:
                with nc.allow_low_precision(reason="bf16 attn"):
                    nc.vector.tensor_scalar(a_t[:], src, rc, None, OP.mult)
                load["dve"] += 200

        def transp_one(h, qt):
            a_t = a_t_live.pop((h, qt))
            prow = slice(64 * (h % 2), 64 * (h % 2) + 64)
            nc.tensor.transpose(trp_ps[prow, qt % 2, :], a_t[:], id_sb[:])
            if qt < THI:
                dst = attnT_hi[prow, h // 2, 128 * qt:128 * (qt + 1)]
                scl = 1.0
            else:
                dst = attnT8[prow, h // 2, 128 * qt:128 * (qt + 1)]
                scl = ATS
            with nc.allow_low_precision(reason="attnT write"):
                if load["act"] <= load["dve"]:
                    nc.scalar.mul(dst, trp_ps[prow, qt % 2, :], scl)
                    load["act"] += 260
                else:
                    nc.vector.tensor_scalar(dst, trp_ps[prow, qt % 2, :], scl,
                                            None, OP.mult)
                    load["dve"] += 260

        def out_st(st, ob):
            hi = st < THI
            for ncb in range(2):
                po = ps4.tile([128, 512], F32, tag="ps512")
                if hi:
                    for kt2 in range(2):
                        nc.tensor.matmul(
                            po[:], attnT_hi[:, kt2, 128 * st:128 * (st + 1)],
                            wo_hi[:, kt2, 512 * ncb:512 * (ncb + 1)],
                            start=(kt2 == 0), stop=(kt2 == 1))
                else:
                    nc.tensor.matmul(po[:],
                                     attnT8[:, :, 128 * st:128 * (st + 1)],
                                     wo8_sb[:, :, 512 * ncb:512 * (ncb + 1)],
                                     start=True, stop=True, perf_mode=DR)
                osc = (1.0 / WS) if hi else (1.0 / OUT_DIV)
                with nc.allow_low_precision(reason="bf16 out"):
                    if load["act"] <= load["dve"]:
                        nc.scalar.mul(ob[:, st % 2, ncb, :], po[:], osc)
                        load["act"] += 580
                    else:
                        nc.vector.tensor_scalar(ob[:, st % 2, ncb, :], po[:],
                                                osc, None, OP.mult)
                        load["dve"] += 670
            if st % 2 == 1:
                dst = aps["out"][256 * (st // 2):256 * (st // 2 + 1), :]
                dma(dst.rearrange("(s p) (n c) -> p s n c", s=2, n=2), ob[:])

        pending = []

        def finish_group(h, qg):
            qsl = slice(4 * qg, 4 * qg + 4)
            with nc.allow_low_precision(reason="recip"):
                nc.vector.reciprocal(recip_sb[:, h % 2, qsl],
                                     den_ps[:, qsl, 0])
            load["dve"] += 180
            for qt in range(4 * qg, 4 * qg + 4):
                norm_tile(h, qt)
            pending.append((h, qg))

        def flush_pending():
            while pending:
                h, qg = pending.pop(0)
                ob = None
                for qt in range(4 * qg, 4 * qg + 4):
                    transp_one(h, qt)
                    if h == 3:
                        if qt % 2 == 0:
                            ob = sm.tile([128, 2, 2, 512], BF16, tag="ob",
                                         bufs=2)
                        out_st(qt, ob)

        def attention(h):
            hp = slice(64 * (h % 2), 64 * (h % 2) + 64)
            pr = h // 2
            tp = (64 * (h % 2), 0)
            for t in range(16):
                flush_pending()
                with tc.high_priority(offset=-100000):
                    for _ in range(2):
                        if fillers:
                            fillers.pop(0)()
                base = 128 * t
                L = S - base
                off = 0
                while off < L:
                    n = min(512, L - off)
                    sc = ps4.tile([128, 512], F32, tag="ps512")
                    nc.tensor.matmul(sc[:, 0:n], kT_bf[hp, pr, base:base + 128],
                                     qT_bf[hp, pr,
                                           base + off:base + off + n],
                                     start=True, stop=True, tile_position=tp)
                    ex = xp.tile([128, 512], BF16, tag="exp")
                    exp_chunk(sc, ex, n, diag=(off == 0),
                              force_act=(base + off < QHI))
                    jorder = list(range(n // 128))
                    if off == 0 and t > 0 and len(jorder) > 1:
                        jorder = jorder[1:] + [0]
                    for j in jorder:
                        qt = t + (off // 128) + j
                        exj = ex[:, 128 * j:128 * (j + 1)]
                        vsrc = (v_hi[:, t, h, :] if t < THI
                                else v_sb[:, t, h, :])
                        nc.tensor.matmul(at_ps[:, qt, :], exj, vsrc,
                                         start=(t == 0 and qt % 8 == 0),
                                         stop=(t == qt), skip_group_check=True)
                        nc.tensor.matmul(den_ps[:, qt, :], exj, ones_sb[:],
                                         start=(t == 0 and qt == 0),
                                         stop=(t == qt), skip_group_check=True)
                    off += n
                if t % 4 == 3:
                    finish_group(h, t // 4)

        for h in range(4):
            attention(h)
        flush_pending()
        while fillers:
            fillers.pop(0)()


_CACHE = {}


def _build():
    if "nc" in _CACHE:
        return _CACHE["nc"], _CACHE["aps"]
    nc = bacc.Bacc("TRN2", target_bir_lowering=False, debug=False,
                   enable_asserts=False, num_devices=N_CORES)
    aps = {
        "xT_bf": nc.dram_tensor("xT_bf", [128, 8, S], BF16, kind="ExternalInput").ap(),
        "xT8": nc.dram_tensor("xT8", [128, 4, 2, S], FP8, kind="ExternalInput").ap(),
        "wq": nc.dram_tensor("wq", [128, 8, 2, 128], BF16, kind="ExternalInput").ap(),
        "wk": nc.dram_tensor("wk", [128, 8, 2, 128], BF16, kind="ExternalInput").ap(),
        "wv8": nc.dram_tensor("wv8", [128, 4, 2, DPC], FP8, kind="ExternalInput").ap(),
        "wo8": nc.dram_tensor("wo8", [128, 2, D], FP8, kind="ExternalInput").ap(),
        "wv_hi": nc.dram_tensor("wv_hi", [128, 8, DPC], BF16, kind="ExternalInput").ap(),
        "wo_hi": nc.dram_tensor("wo_hi", [128, 2, D], BF16, kind="ExternalInput").ap(),
        "cosT": nc.dram_tensor("cosT", [128, S], BF16, kind="ExternalInput").ap(),
        "sinT": nc.dram_tensor("sinT", [128, S], BF16, kind="ExternalInput").ap(),
        "ident": nc.dram_tensor("ident", [128, 128], BF16, kind="ExternalInput").ap(),
        "dmask": nc.dram_tensor("dmask", [128, 128], BF16, kind="ExternalInput").ap(),
        "out": nc.dram_tensor("out", [S, D], BF16, kind="ExternalOutput").ap(),
    }
    with tile.TileContext(nc) as tc:
        _emit(tc, aps)
    nc.compile()
    _CACHE["nc"], _CACHE["aps"] = nc, aps
    return nc, aps


def _host_tables():
    """cos/sin [128, S] bf16: partition p = 64*(h%2)+dk, scaled by 1/16."""
    p = np.arange(128)
    pos = np.arange(S, dtype=np.float64)
    dk_idx = p % 64
    freq = THETA ** (-2.0 * (dk_idx // 2) / DK)
    ang = pos[None, :] * freq[:, None]
    cosT = (np.cos(ang) / WS).astype(np.float32)
    sgn = np.where(dk_idx % 2 == 0, -1.0, 1.0)
    sinT = (sgn[:, None] * np.sin(ang) / WS).astype(np.float32)
    return (cosT.astype(ml_dtypes.bfloat16), sinT.astype(ml_dtypes.bfloat16))


def _pack_wqk(Wc):
    """Wc [256, 1024] -> [128(p), 8(kt), 2(mt=pair), 128(m)] bf16 (x16)."""
    arr = (Wc * WS).reshape(2, 128, 8, 128)            # [mt, m, kt, p]
    return np.ascontiguousarray(
        arr.transpose(3, 2, 0, 1)).astype(ml_dtypes.bfloat16)


def make_in_maps(x, Wq, Wk, Wv, Wo):
    cosT, sinT = _host_tables()
    ident = np.eye(128, dtype=ml_dtypes.bfloat16)
    dmask = np.triu(np.ones((128, 128), ml_dtypes.bfloat16))  # keep sq >= sk
    xT_bf, xT8 = [], []
    for b in range(B):
        xt = x[b].T
        xT_bf.append(np.ascontiguousarray(
            xt.reshape(8, 128, S).transpose(1, 0, 2)).astype(ml_dtypes.bfloat16))
        xT8.append(np.ascontiguousarray(
            xt.reshape(4, 2, 128, S).transpose(2, 0, 1, 3)).astype(
                ml_dtypes.float8_e4m3))
    maps = []
    for c in range(N_CORES):
        b, g = c // 4, c % 4
        rows = slice(DPC * g, DPC * (g + 1))
        wv8 = np.ascontiguousarray(
            (Wv[rows, :].T * WS).reshape(4, 2, 128, DPC).transpose(2, 0, 1, 3))
        wo_s = np.ascontiguousarray(
            (Wo[:, rows].T * WS).reshape(2, 128, D).transpose(1, 0, 2))
        wv_hi = np.ascontiguousarray(
            (Wv[rows, :].T * WS).reshape(8, 128, DPC).transpose(1, 0, 2))
        maps.append({
            "xT_bf": xT_bf[b],
            "xT8": xT8[b],
            "wq": _pack_wqk(Wq[rows, :]),
            "wk": _pack_wqk(Wk[rows, :]),
            "wv8": wv8.astype(ml_dtypes.float8_e4m3),
            "wo8": wo_s.astype(ml_dtypes.float8_e4m3),
            "wv_hi": wv_hi.astype(ml_dtypes.bfloat16),
            "wo_hi": wo_s.astype(ml_dtypes.bfloat16),
            "cosT": cosT,
            "sinT": sinT,
            "ident": ident,
            "dmask": dmask,
        })
    return maps


def kernel(x, Wq, Wk, Wv, Wo, _trace=False, _tmpdir=None):
    x, Wq, Wk, Wv, Wo = (np.asarray(a, dtype=np.float32)
                         for a in (x, Wq, Wk, Wv, Wo))
    nc, _ = _build()
    maps = make_in_maps(x, Wq, Wk, Wv, Wo)
    res = run_bass_kernel_spmd(nc, maps, core_ids=list(range(N_CORES)),
                               trace=_trace, tmpdir=_tmpdir)
    out = np.zeros((B, S, D), np.float32)
    for c in range(N_CORES):
        out[c // 4] += res.results[c]["out"].astype(np.float32)
    if _trace:
        kernel.last_results = res
    return out
